# revision 68
# baseline (speedup 1.0000x reference)
"""Distributed causal-attention kernel for one TRN2 chip (8 NeuronCores).

Sharding (hardcoded): core i handles batch i//4 and head-group i%4
(2 heads of 8, head_dim 128).  Each core:
  RMSNorm(x_b) -> QKV proj (its heads) -> causal attention -> O^T
  -> partial output projection y^T_partial = sum_h Wout_h^T O_h^T
     for the FULL sequence of its batch (contribution of its 2 heads).
Host folds gamma + attention scale into the pre-transposed weights and
sums the 4 per-head-group partials of each batch during unsharding
(the reduction IS the gather for row-split to_out).  No collectives.
"""

import numpy as np

import concourse.mybir as mybir
import concourse.tile as tile
from concourse import bacc
from concourse.bass_utils import run_bass_kernel_spmd
from concourse.masks import make_identity

F32 = mybir.dt.float32
F32R = mybir.dt.float32r
F16 = mybir.dt.float16
BF = mybir.dt.bfloat16
AF = mybir.ActivationFunctionType

S = 2048          # sequence length
D = 1024          # model dim
DH = 128          # head dim
HC = 2            # heads per core
FQKV = 3 * HC * DH  # 768 qkv cols per core (pre-transposed layout)
P = 128
SB = S // P       # 16 seq blocks
KD = D // P       # 8 d blocks
SA = float(DH) ** -0.5


def _body(tc):
    nc = tc.nc
    x_ext = nc.declare_dram_parameter("x", [S, D], BF, isOutput=False)
    wqkv_ext = nc.declare_dram_parameter("w_qkvT", [D, FQKV], BF, isOutput=False)
    wout_ext = nc.declare_dram_parameter("w_outT", [DH, HC, D], BF, isOutput=False)
    out_ext = nc.declare_dram_parameter("out", [D, S], F16, isOutput=True)

    from contextlib import ExitStack
    with ExitStack() as ctx:
        wpool = ctx.enter_context(tc.tile_pool(name="wpool", bufs=1))
        wqkvT = wpool.tile([P, KD, FQKV], BF)
        nc.scalar.dma_start(
            wqkvT, wqkv_ext.ap().rearrange("(o p) f -> p o f", p=P))
        woT = wpool.tile([P, HC, D], BF)

        const = ctx.enter_context(tc.tile_pool(name="const", bufs=1))
        big = ctx.enter_context(tc.tile_pool(name="big", bufs=1))
        cast = ctx.enter_context(tc.tile_pool(name="cast", bufs=6))
        stat = ctx.enter_context(tc.tile_pool(name="stat", bufs=8))
        ptp = ctx.enter_context(tc.tile_pool(name="ptp", bufs=10))
        lacc = ctx.enter_context(tc.tile_pool(name="lacc", bufs=6))
        yout = ctx.enter_context(tc.tile_pool(name="yout", bufs=6))
        ps_mm = ctx.enter_context(tc.tile_pool(name="ps_mm", bufs=2, space="PSUM"))
        ps_s = ctx.enter_context(tc.tile_pool(name="ps_s", bufs=3, space="PSUM"))
        ps_o = ctx.enter_context(tc.tile_pool(name="ps_o", bufs=2, space="PSUM"))
        ps_l = ctx.enter_context(tc.tile_pool(name="ps_l", bufs=1, space="PSUM"))

        # ---- constants ----
        ident = const.tile([P, P], BF)
        make_identity(nc, ident)
        # force both activation-table loads into the startup DMA window
        warm = const.tile([1, 2], F32)
        nc.vector.memset(warm, 1.0)
        nc.scalar.activation(warm[:, 0:1], warm[:, 0:1], AF.Sqrt)
        nc.scalar.activation(warm[:, 1:2], warm[:, 1:2], AF.Exp)

        # ---- persistent activations ----
        xnT = big.tile([P, KD, S], BF)
        qkvT = big.tile([P, 6, S], BF)
        v_sb = big.tile([P, SB, HC * DH], BF)
        oS = big.tile([P, HC, 4, 512], BF)   # O^T per (head, seq superblock)

        ones_h = const.tile([P, 1], F16)
        nc.vector.memset(ones_h, 1.0)

        # ---- phase 1 (software-pipelined): transpose raw x -> norm in
        # place -> QKV -> V.  ssq/scale of chunk c+1 is issued ahead of
        # QKV(c) so the PE never waits on the norm chain mid-stream.
        def x_transp(c):
            # raw x^T straight from DRAM (no upstream dependency)
            for k in range(KD):
                nc.sync.dma_start_transpose(
                    xnT[:, k, c * 512:(c + 1) * 512],
                    x_ext[c * 512:(c + 1) * 512, k * P:(k + 1) * P])

        for c in range(4):
            x_transp(c)

        def norm_stats(c):
            # ssq over d (partition axis) via squares + ones-matmuls
            ssqp = ps_l.tile([1, 512], F32, tag="lp", name=f"ssq{c}")
            for k in range(KD):
                xsq = cast.tile([P, 512], F16, tag="xsq")
                nc.vector.tensor_mul(xsq, xnT[:, k, c * 512:(c + 1) * 512],
                                     xnT[:, k, c * 512:(c + 1) * 512])
                nc.tensor.matmul(ssqp, ones_h, xsq,
                                 start=(k == 0), stop=(k == KD - 1))
            # row scale = 32/sqrt(ssq) = 1/sqrt(ssq/1024)
            srt = stat.tile([1, 512], F32, tag="srt")
            nc.scalar.activation(srt, ssqp, AF.Sqrt, scale=1.0 / D)
            scr = stat.tile([1, 512], F32, tag="scr")
            nc.vector.reciprocal(scr, srt)
            scb = cast.tile([P, 512], F32, tag="scb")
            nc.gpsimd.partition_broadcast(scb, scr)
            for k in range(KD):
                eng = nc.vector if k % 2 == 0 else nc.gpsimd
                eng.tensor_mul(xnT[:, k, c * 512:(c + 1) * 512],
                               xnT[:, k, c * 512:(c + 1) * 512], scb)

        def qkv_chunk(c):
            # QKV projection for this chunk (pairs share ldweights)
            for fp in range(3):
                pms = [ps_mm.tile([P, 512], F32, tag="pm", name=f"pm{u}")
                       for u in range(2)]
                for k in range(KD):
                    for u in range(2):
                        fb = fp * 2 + u
                        nc.tensor.matmul(
                            pms[u], wqkvT[:, k, fb * P:(fb + 1) * P],
                            xnT[:, k, c * 512:(c + 1) * 512],
                            start=(k == 0), stop=(k == KD - 1))
                for u in range(2):
                    fb = fp * 2 + u
                    if u == 0:
                        nc.vector.tensor_copy(
                            qkvT[:, fb, c * 512:(c + 1) * 512], pms[u])
                    else:
                        nc.scalar.activation(
                            qkvT[:, fb, c * 512:(c + 1) * 512], pms[u],
                            AF.Copy)

        def v_nat(c):
            # V natural layout for this chunk
            for h in range(HC):
                pst = ps_mm.tile([P, 512], BF, tag="pm")
                for j in range(4):
                    sb = c * 4 + j
                    nc.tensor.transpose(
                        pst[:, j * P:(j + 1) * P],
                        qkvT[:, 4 + h, sb * P:(sb + 1) * P], ident)
                nc.vector.tensor_copy(
                    v_sb[:, c * 4:(c + 1) * 4, h * DH:(h + 1) * DH],
                    pst.rearrange("p (j q) -> p j q", j=4))

        norm_stats(0)
        norm_stats(1)
        qkv_chunk(0)
        norm_stats(2)
        qkv_chunk(1)
        v_nat(0)
        norm_stats(3)
        qkv_chunk(2)
        v_nat(1)
        qkv_chunk(3)
        v_nat(2)
        v_nat(3)

        # deferred: w_out slice not needed until the first out-projection
        nc.scalar.dma_start(woT, wout_ext.ap())

        # ---- attention: S^T = K^T-block x Q^T -> exp -> PV, 1/l at the end
        ones_f = const.tile([P, 1], F32)
        nc.vector.memset(ones_f, 1.0)
        ones_r = const.tile([P, 1], F32R)
        nc.vector.tensor_copy(ones_r, ones_f)
        # persistent diag-block prob tiles: cols [0, t*128) stay zero forever
        dZ = big.tile([P, 4, HC, 512], BF)
        for t in range(1, 4):
            for h in range(HC):
                nc.gpsimd.memset(dZ[:, t, h, : t * P], 0.0)

        def attn_super(h, a):
            po = ps_o.tile([P, 512], F32, tag="po", name=f"po{h}_{a}")
            # dual l-accumulators: even jb on DVE, odd jb on Pool
            la = [lacc.tile([P, 512], F32R, tag=f"la{u}", name=f"la{u}_{h}_{a}")
                  for u in range(2)]
            nj = 4 * (a + 1)
            pend = None

            def pv(jb, lo, ptt):
                nc.tensor.matmul(
                    po[:, lo:], v_sb[:, jb, h * DH:(h + 1) * DH], ptt[:, lo:],
                    start=(jb == 0), stop=(jb == nj - 1),
                    skip_group_check=True)

            for jb in range(nj):
                t = jb - 4 * a
                lo = t * P if t > 0 else 0   # causally-live column range start
                ps = ps_s.tile([P, 512], F32, tag="s", name=f"ps{h}_{a}_{jb}")
                nc.tensor.matmul(
                    ps[:, lo:], qkvT[:, 2 + h, jb * P:(jb + 1) * P],
                    qkvT[:, h, a * 512 + lo:(a + 1) * 512],
                    start=True, stop=True)
                if pend is not None:
                    pv(*pend)
                if t >= 0:
                    ptt = dZ[:, t, h]
                else:
                    ptt = ptp.tile([P, 512], BF, tag="ptt",
                                   name=f"ptt{h}_{a}_{jb}")
                nc.scalar.activation(ptt[:, lo:], ps[:, lo:], AF.Exp)
                if t >= 0:
                    # causal zeroing after exp (cheaper than masking logits)
                    nc.gpsimd.affine_select(
                        out=ptt[:, t * P:(t + 1) * P],
                        in_=ptt[:, t * P:(t + 1) * P],
                        compare_op=mybir.AluOpType.is_ge,
                        fill=0.0, base=0,
                        pattern=[[1, P]], channel_multiplier=-1)
                eng = nc.vector if jb % 2 == 0 else nc.gpsimd
                if jb < 2:
                    eng.tensor_copy(la[jb % 2], ptt)
                else:
                    eng.tensor_add(la[jb % 2][:, lo:], la[jb % 2][:, lo:],
                                   ptt[:, lo:])
                pend = (jb, lo, ptt)
            pv(*pend)
            lp = ps_l.tile([1, 512], F32, tag="lp", name=f"lp{h}_{a}")
            nc.tensor.matmul(lp, ones_r, la[0], start=True, stop=False)
            nc.tensor.matmul(lp, ones_r, la[1], start=False, stop=True)
            rl = stat.tile([1, 512], BF, tag="rl")
            with nc.allow_low_precision(reason="bf16 1/l bcast"):
                nc.vector.reciprocal(rl, lp)
            rlb = cast.tile([P, 512], BF, tag="rlb")
            nc.gpsimd.partition_broadcast(rlb, rl)
            nc.vector.tensor_mul(oS[:, h, a], po, rlb)

        def out_proj(a):
            # partial out-proj for this 512-seq chunk: y^T[c,:] += sum_h
            for cp in range(4):
                pms = [ps_mm.tile([P, 512], F32, tag="pm", name=f"pmo{u}")
                       for u in range(2)]
                for u in range(2):
                    cb = cp * 2 + u
                    for h in range(HC):
                        nc.tensor.matmul(
                            pms[u], woT[:, h, cb * P:(cb + 1) * P],
                            oS[:, h, a],
                            start=(h == 0), stop=(h == HC - 1))
                for u in range(2):
                    cb = cp * 2 + u
                    y = yout.tile([P, 512], F16, tag="y")
                    nc.vector.tensor_copy(y, pms[u])
                    nc.sync.dma_start(
                        out_ext[cb * P:(cb + 1) * P,
                                a * 512:(a + 1) * 512], y)

        # a-outer / h-inner; out_proj(a) delayed one superblock so the PE
        # never waits on a's 1/l chain before starting a+1's attention
        for a in range(4):
            for h in range(HC):
                attn_super(h, a)
            if a > 0:
                out_proj(a - 1)
        out_proj(3)


def build():
    nc = bacc.Bacc(None, target_bir_lowering=False)
    with tile.TileContext(nc) as tc:
        _body(tc)
    nc.compile()
    return nc


_NC = None


def make_in_maps(inputs):
    import ml_dtypes
    x = np.ascontiguousarray(np.asarray(inputs["x"], np.float32))
    gamma = np.asarray(inputs["gamma"], np.float32)
    w_qkv = np.asarray(inputs["w_qkv"], np.float32)
    w_out = np.asarray(inputs["w_out"], np.float32)
    w_prep = w_qkv * gamma[None, :]          # fold RMSNorm gamma
    in_maps = []
    for i in range(8):
        b, g = i // 4, i % 4
        rows = np.concatenate([
            w_prep[256 * g:256 * (g + 1)] * SA,   # fold attn scale into Q
            w_prep[1024 + 256 * g:1024 + 256 * (g + 1)],
            w_prep[2048 + 256 * g:2048 + 256 * (g + 1)]], axis=0)
        # w_out columns for this head group, laid out [dh, h, c_out]
        wo = w_out[:, 256 * g:256 * (g + 1)].T.reshape(HC, DH, D)
        wo = np.ascontiguousarray(wo.transpose(1, 0, 2))
        in_maps.append({
            "x": np.ascontiguousarray(x[b]).astype(ml_dtypes.bfloat16),
            "w_qkvT": np.ascontiguousarray(rows.T).astype(ml_dtypes.bfloat16),
            "w_outT": wo.astype(ml_dtypes.bfloat16)})
    return in_maps


def run(inputs, trace=False):
    global _NC
    if _NC is None:
        _NC = build()
    in_maps = make_in_maps(inputs)
    br = run_bass_kernel_spmd(_NC, in_maps, list(range(8)), trace=trace)
    out = np.empty((2, S, D), np.float32)
    for b in range(2):
        acc = np.zeros((D, S), np.float32)
        for g in range(4):
            acc += np.asarray(br.results[4 * b + g]["out"], np.float32)
        out[b] = acc.T
    return out, br


def kernel(**inputs):
    out, _ = run(inputs, trace=False)
    return out
